# revision 1
# baseline (speedup 1.0000x reference)
"""CLIP contrastive loss on 8 Trainium2 NeuronCores (Bass/Tile), fp8 DoubleRow.

Strategy (data-parallel over image rows, hint's local_loss path):
  - Core c holds image rows [c*1024, (c+1)*1024) and the FULL text matrix.
  - Text rows are rolled by c*1024 on the host so every core's diagonal
    block sits at local cols [0, 1024) (the compiled program is
    core-independent).
  - Features are scaled by 16 on the host and quantized to fp8 e4m3; the
    PE runs DoubleRow matmuls (2 fp8 k-planes per cell, K=256 per MM,
    out width 512 = the s3d3 ISA max) at 2x bf16 throughput: 250 MMs at
    the warm 216ns back-to-back rate ~= 55us of pure PE time per core.
  - Loop: for each 2048-col group (4 of them), for each 128-row m-tile
    (8): 2 kc x 4 DoubleRow MMs -> [128, 2048] PSUM (4 banks, 2-deep
    ping-pong), then ONE ACT exp over the whole 4-bank span (bf16 out,
    accum_out = row-sum partials) and a DVE tensor_add into the group's
    column accumulator.  ACT is the steady-state gate at ~2.2us/slot
    (exp streams 1 elem/cycle/lane at 1.2GHz + READ_ACCUMULATOR).
  - Diagonals all live in group 0 (cols mt*128..mt*128+128): DVE
    tensor_mul with (scale/256)*I + free-axis reduce.
  - Input DMAs are ordered/chunked so the first MM starts after only
    32KB of image + 128KB of text arrive.
  - Host: partition-reduce column accumulators, combine row/col exp-sums
    and diagonals in float64: lse = shift + log(sum); mean over both
    directions.

Fixed-shift logsumexp is numerically safe: logits are bounded by +-scale
and shift = scale/2 keeps every term that matters in normal f32 range.

Measured (rel err 5.2e-4 vs 2e-2 tolerance): ~95.8us HW exec vs 133.4us
bf16 baseline.  Things measured NOT to help on this runtime: DVE exp
offload via int16-Schraudolph (tensor_scalar is 1x from f32 PSUM and its
PSUM hold stalls the PE), tensor_tensor_reduce / affine_mul_reduce
(device-unsupported), scalar_tensor_tensor accum fusion (1x rate, and
accum_out is wrong when out aliases in1), rowsums via DVE tensor_reduce
(1x), MM_W=1024 (ISA rejects >1024 fp8 moving elements).
"""

from contextlib import ExitStack

import numpy as np
import ml_dtypes

import concourse.bass as bass
from concourse import bacc
import concourse.tile as tile
from concourse import mybir
from concourse.bass import ts
from concourse.bass_utils import run_bass_kernel_spmd

N = 8192
D = 512
NC = 8
M_LOC = N // NC          # 1024 image rows per core
MT = M_LOC // 128        # 8 m-tiles of 128 rows
NG = 4                   # column groups
GW = N // NG             # 2048 cols per group
KC = 2                   # DoubleRow K-chunks (256 each)
PRE = 16.0               # host-side fp8 pre-scale per operand

F32 = mybir.dt.float32
BF16 = mybir.dt.bfloat16
FP8 = mybir.dt.float8e4

# moving-free per matmul: out width = MM_W, moving fp8 elements = 2*MM_W
MM_W = 512

_CACHE = {}
LAST_RESULTS = None


def _build(scale: float, shift: float, mm_w: int):
    n_mm = GW // mm_w            # matmuls per (kc, group-span)
    act_scale = scale / (PRE * PRE)
    nc = bacc.Bacc("TRN2", debug=False)

    at_d = nc.dram_tensor("at_in", [128, KC, 2, M_LOC], FP8, kind="ExternalInput").ap()
    bt_d = nc.dram_tensor("bt_in", [NG, KC, 128, 2, GW], FP8, kind="ExternalInput").ap()
    eye_d = nc.dram_tensor("eye_in", [128, 128], F32, kind="ExternalInput").ap()

    rowr_d = nc.dram_tensor("rowr_out", [128, MT, NG], F32, kind="ExternalOutput").ap()
    colsum_d = nc.dram_tensor("colsum_out", [NG, 128, GW], BF16, kind="ExternalOutput").ap()
    diag_d = nc.dram_tensor("diag_out", [128, MT], F32, kind="ExternalOutput").ap()

    with ExitStack() as ctx:
        tc = ctx.enter_context(tile.TileContext(nc))
        singles = ctx.enter_context(tc.tile_pool(name="singles", bufs=1))
        btp = ctx.enter_context(tc.tile_pool(name="btp", bufs=NG * KC))
        expp = ctx.enter_context(tc.tile_pool(name="expp", bufs=3))
        cap = ctx.enter_context(tc.tile_pool(name="cap", bufs=2))
        scr = ctx.enter_context(tc.tile_pool(name="scr", bufs=2))
        psum = ctx.enter_context(tc.tile_pool(name="psum", bufs=2, space="PSUM"))

        at_t = singles.tile([128, KC, 2, M_LOC], FP8)
        bt_tiles = [
            [btp.tile([128, 2, GW], FP8, name=f"bt{g}_{kc}", tag="bt") for kc in range(KC)]
            for g in range(NG)
        ]
        # Load order tuned for fastest first-matmul: the first MM needs only
        # at[mt0, kc0] (32KB) and bt00's first 512 cols (128KB); stream the
        # rest in consumption order.
        nc.sync.dma_start(at_t[:, 0, :, 0:128], at_d[:, 0, :, 0:128])
        nc.sync.dma_start(bt_tiles[0][0][:, :, 0:512], bt_d[0, 0][:, :, 0:512])
        nc.sync.dma_start(at_t[:, 0, :, 128:M_LOC], at_d[:, 0, :, 128:M_LOC])
        nc.sync.dma_start(bt_tiles[0][0][:, :, 512:1024], bt_d[0, 0][:, :, 512:1024])
        nc.sync.dma_start(bt_tiles[0][0][:, :, 1024:GW], bt_d[0, 0][:, :, 1024:GW])
        nc.sync.dma_start(at_t[:, 1], at_d[:, 1])
        for h in range(2):
            nc.sync.dma_start(
                bt_tiles[0][1][:, :, ts(h, GW // 2)], bt_d[0, 1][:, :, ts(h, GW // 2)]
            )
        eye_t = singles.tile([128, 128], F32)
        nc.sync.dma_start(eye_t, eye_d)
        bias_t = singles.tile([128, 1], F32)
        nc.vector.memset(bias_t, -shift)
        # Dummy 1-element exp pre-loads the ACT table set (~1.3us) during
        # the initial DMA wait instead of on the first real slot.
        warm_t = singles.tile([128, 1], F32)
        nc.scalar.activation(
            warm_t, bias_t, mybir.ActivationFunctionType.Exp, bias=0.0, scale=1.0
        )
        rowr = singles.tile([128, MT, NG], F32)
        diag_sb = singles.tile([128, MT], F32)

        for g in range(1, NG):
            for kc in range(KC):
                nc.sync.dma_start(bt_tiles[g][kc], bt_d[g, kc])

        for g in range(NG):
            colacc = cap.tile([128, GW], BF16, name=f"cacc{g}", tag="cacc")
            for mt in range(MT):
                s_ps = psum.tile([128, GW], F32, name=f"s{g}_{mt}", tag="spsum")
                for kc in range(KC):
                    lhsT = at_t[:, kc, :, ts(mt, 128)]          # [128, 2, 128]
                    for w in range(n_mm):
                        nc.tensor.matmul(
                            s_ps[:, ts(w, mm_w)],
                            lhsT,
                            bt_tiles[g][kc][:, :, ts(w, mm_w)],  # [128, 2, mm_w]
                            start=(kc == 0),
                            stop=(kc == KC - 1),
                            perf_mode=mybir.MatmulPerfMode.DoubleRow,
                        )
                if g == 0:
                    # diag block for mt sits at local cols [mt*128, mt*128+128)
                    dscr = scr.tile([128, 128], F32, name=f"dscr{mt}", tag="dscr")
                    nc.vector.tensor_mul(dscr, s_ps[:, ts(mt, 128)], eye_t)
                    nc.vector.tensor_reduce(
                        out=diag_sb[:, mt : mt + 1],
                        in_=dscr,
                        axis=mybir.AxisListType.X,
                        op=mybir.AluOpType.add,
                    )
                e_t = expp.tile([128, GW], BF16, name=f"e{g}_{mt}", tag="exp")
                nc.scalar.activation(
                    e_t,
                    s_ps,
                    mybir.ActivationFunctionType.Exp,
                    bias=bias_t,
                    scale=act_scale,
                    accum_out=rowr[:, mt, g : g + 1],
                )
                if mt == 0:
                    nc.vector.tensor_copy(colacc, e_t)
                else:
                    nc.vector.tensor_add(colacc, colacc, e_t)
            nc.sync.dma_start(colsum_d[g], colacc)
            if g == 0:
                # diag is complete after group 0; stream it out early.
                nc.sync.dma_start(diag_d, diag_sb)

        nc.sync.dma_start(rowr_d, rowr)

    nc.compile()
    return nc


def _prep_inputs(img, txt, scale):
    fp8 = ml_dtypes.float8_e4m3fn
    eye = ((scale / (PRE * PRE)) * np.eye(128)).astype(np.float32)
    in_maps = []
    for c in range(NC):
        A = (PRE * img[c * M_LOC : (c + 1) * M_LOC]).astype(fp8)   # [1024, 512]
        # k = kc*256 + ko*128 + p
        at = np.ascontiguousarray(
            A.T.reshape(KC, 2, 128, M_LOC).transpose(2, 0, 1, 3)
        )                                                          # [128, KC, 2, 1024]
        tr = np.roll(txt, -c * M_LOC, axis=0)                      # local col j -> global (j + c*1024) % N
        B = (PRE * tr).astype(fp8)                                 # [8192, 512]
        bt = np.ascontiguousarray(
            B.T.reshape(KC, 2, 128, NG, GW).transpose(3, 0, 2, 1, 4)
        )                                                          # [NG, KC, 128, 2, GW]
        in_maps.append({"at_in": at, "bt_in": bt, "eye_in": eye})
    return in_maps


def kernel(image_features, text_features, logit_scale):
    global LAST_RESULTS
    img = np.ascontiguousarray(np.asarray(image_features, dtype=np.float32))
    txt = np.ascontiguousarray(np.asarray(text_features, dtype=np.float32))
    scale = float(np.asarray(logit_scale))
    shift = 0.5 * scale

    key = (scale, MM_W)
    if key not in _CACHE:
        _CACHE[key] = _build(scale, shift, MM_W)
    nc = _CACHE[key]

    in_maps = _prep_inputs(img, txt, scale)
    res = None
    last_err = None
    for _attempt in range(3):
        try:
            res = run_bass_kernel_spmd(nc, in_maps, core_ids=list(range(NC)))
            break
        except Exception as e:  # transient NRT/device hiccups: retry
            last_err = e
    if res is None:
        raise last_err
    LAST_RESULTS = res

    colsum_tot = np.zeros(N, dtype=np.float64)
    lse_rows = []
    diags = []
    for c, r in enumerate(res.results):
        rowsum = r["rowr_out"].astype(np.float64).sum(axis=2)       # [128, MT]
        lse_rows.append(shift + np.log(rowsum.T.reshape(-1)))       # row = mt*128 + p
        diags.append(r["diag_out"].astype(np.float64).T.reshape(-1))
        colsum_tot += np.roll(
            r["colsum_out"].astype(np.float64).sum(axis=1).reshape(-1), c * M_LOC
        )
    lse_row = np.concatenate(lse_rows)
    diag = np.concatenate(diags)
    lse_col = shift + np.log(colsum_tot)

    loss = 0.5 * (np.mean(lse_row - diag) + np.mean(lse_col - diag))
    return np.float32(loss)



# revision 2
# speedup vs baseline: 1.0316x; 1.0316x over previous
"""CLIP contrastive loss on 8 Trainium2 NeuronCores (Bass/Tile), fp8 DoubleRow.

Strategy (data-parallel over image rows, hint's local_loss path):
  - Core c holds image rows [c*1024, (c+1)*1024) and the FULL text matrix.
  - Text rows are rolled by c*1024 on the host so every core's diagonal
    block sits at local cols [0, 1024) (the compiled program is
    core-independent).
  - Features are scaled by 16 on the host and quantized to fp8 e4m3; the
    PE runs DoubleRow matmuls (2 fp8 k-planes per cell, K=256 per MM,
    out width 512 = the s3d3 ISA max) at 2x bf16 throughput.
  - Loop: for each 2048-col group (4 of them), for each 128-row m-tile
    (8): 2 kc x 4 DoubleRow MMs -> [128, 2048] PSUM (4 banks, 2-deep
    ping-pong), then ONE ACT exp over the whole 4-bank span (bf16 out,
    accum_out = row-sum partials) and a DVE tensor_add into the group's
    column accumulator.  ACT is the steady-state gate at ~2.04us/slot
    (exp streams 1 elem/cycle/lane at 1.2GHz + READ_ACCUMULATOR).
  - Startup care: `at` is staged [128, MT, KC, 2, 128] so every DMA is
    contiguous >=512B/partition; the critical prefix (at mt0 + the left
    halves of bt[0][*]) is 640KB and is issued first on the HWDGE FIFO;
    g0/mt0 is processed as two [128,1024] EXPs so the first EXP fires as
    soon as the left halves land (~12us vs 19.4us unsplit).
  - Tail care: g3/mt7 is also split in halves so the last colacc ADD and
    colsum DMA overlap the last EXP; colsum for g3 is written out in two
    half DMAs.
  - Diagonals all live in group 0 (cols mt*128..mt*128+128): DVE
    tensor_mul with (scale/256)*I + free-axis reduce.
  - Host: partition-reduce column accumulators, combine row/col exp-sums
    and diagonals in float64: lse = shift + log(sum); mean over both
    directions.

Fixed-shift logsumexp is numerically safe: logits are bounded by +-scale
and shift = scale/2 keeps every term that matters in normal f32 range.
"""

from contextlib import ExitStack

import numpy as np
import ml_dtypes

import concourse.bass as bass
from concourse import bacc
import concourse.tile as tile
from concourse import mybir
from concourse.bass import ts
from concourse.bass_utils import run_bass_kernel_spmd

N = 8192
D = 512
NC = 8
M_LOC = N // NC          # 1024 image rows per core
MT = M_LOC // 128        # 8 m-tiles of 128 rows
NG = 4                   # column groups
GW = N // NG             # 2048 cols per group
HW = GW // 2             # half-group width (split slots)
KC = 2                   # DoubleRow K-chunks (256 each)
PRE = 16.0               # host-side fp8 pre-scale per operand
RS = MT + 1              # rowr slots per group (mt0/mt7 splits add one)

F32 = mybir.dt.float32
BF16 = mybir.dt.bfloat16
FP8 = mybir.dt.float8e4

# moving-free per matmul: out width = MM_W, moving fp8 elements = 2*MM_W
MM_W = 512

_CACHE = {}
LAST_RESULTS = None


def _build(scale: float, shift: float, mm_w: int):
    n_mm = GW // mm_w            # matmuls per (kc, group-span)
    act_scale = scale / (PRE * PRE)
    nc = bacc.Bacc("TRN2", debug=False)

    at_d = nc.dram_tensor("at_in", [128, MT, KC, 2, 128], FP8, kind="ExternalInput").ap()
    bt_d = nc.dram_tensor("bt_in", [NG, KC, 128, 2, GW], FP8, kind="ExternalInput").ap()
    eye_d = nc.dram_tensor("eye_in", [128, 128], F32, kind="ExternalInput").ap()

    rowr_d = nc.dram_tensor("rowr_out", [128, NG, RS], F32, kind="ExternalOutput").ap()
    colsum_d = nc.dram_tensor("colsum_out", [NG, 128, GW], BF16, kind="ExternalOutput").ap()
    diag_d = nc.dram_tensor("diag_out", [128, MT], F32, kind="ExternalOutput").ap()

    with ExitStack() as ctx:
        tc = ctx.enter_context(tile.TileContext(nc))
        singles = ctx.enter_context(tc.tile_pool(name="singles", bufs=1))
        btp = ctx.enter_context(tc.tile_pool(name="btp", bufs=NG * KC))
        expp = ctx.enter_context(tc.tile_pool(name="expp", bufs=3))
        cap = ctx.enter_context(tc.tile_pool(name="cap", bufs=2))
        scr = ctx.enter_context(tc.tile_pool(name="scr", bufs=2))
        psum = ctx.enter_context(tc.tile_pool(name="psum", bufs=2, space="PSUM"))

        # Warm-up chain first in program order: nothing here depends on DMA,
        # so the ACT table set loads during NEFF bring-up instead of gating
        # the first real EXP.
        bias_t = singles.tile([128, 1], F32)
        nc.vector.memset(bias_t, -shift)
        warm_t = singles.tile([128, 1], F32)
        nc.scalar.activation(
            warm_t, bias_t, mybir.ActivationFunctionType.Exp, bias=0.0, scale=1.0
        )

        at_t = singles.tile([128, MT, KC, 2, 128], FP8)
        bt_tiles = [
            [btp.tile([128, 2, GW], FP8, name=f"bt{g}_{kc}", tag="bt") for kc in range(KC)]
            for g in range(NG)
        ]
        # Critical-prefix DMA order (HWDGE FIFO = priority order): the first
        # EXP needs only at mt0 (64KB, contiguous) + the left halves of
        # bt[0][0] and bt[0][1] (512KB).  Everything else streams after.
        nc.sync.dma_start(at_t[:, 0], at_d[:, 0])
        nc.sync.dma_start(bt_tiles[0][0][:, :, 0:HW], bt_d[0, 0][:, :, 0:HW])
        nc.sync.dma_start(bt_tiles[0][1][:, :, 0:HW], bt_d[0, 1][:, :, 0:HW])
        nc.sync.dma_start(bt_tiles[0][0][:, :, HW:GW], bt_d[0, 0][:, :, HW:GW])
        nc.sync.dma_start(bt_tiles[0][1][:, :, HW:GW], bt_d[0, 1][:, :, HW:GW])
        nc.sync.dma_start(at_t[:, 1:MT], at_d[:, 1:MT])
        eye_t = singles.tile([128, 128], F32)
        nc.sync.dma_start(eye_t, eye_d)
        for g in range(1, NG):
            for kc in range(KC):
                nc.sync.dma_start(bt_tiles[g][kc], bt_d[g, kc])

        rowr = singles.tile([128, NG, RS], F32)
        diag_sb = singles.tile([128, MT], F32)

        def mm_halves(s_ps, g, mt):
            # w-pair-major order: cols [0:HW) finish after 4 MMs so a half
            # EXP can start while the right half is still multiplying.
            for wpair in range(2):
                for kc in range(KC):
                    lhsT = at_t[:, mt, kc]                     # [128, 2, 128]
                    for w in range(wpair * n_mm // 2, (wpair + 1) * n_mm // 2):
                        nc.tensor.matmul(
                            s_ps[:, ts(w, mm_w)],
                            lhsT,
                            bt_tiles[g][kc][:, :, ts(w, mm_w)],
                            start=(kc == 0),
                            stop=(kc == KC - 1),
                            perf_mode=mybir.MatmulPerfMode.DoubleRow,
                        )

        def mm_full(s_ps, g, mt):
            for kc in range(KC):
                lhsT = at_t[:, mt, kc]                         # [128, 2, 128]
                for w in range(n_mm):
                    nc.tensor.matmul(
                        s_ps[:, ts(w, mm_w)],
                        lhsT,
                        bt_tiles[g][kc][:, :, ts(w, mm_w)],
                        start=(kc == 0),
                        stop=(kc == KC - 1),
                        perf_mode=mybir.MatmulPerfMode.DoubleRow,
                    )

        def do_diag(s_ps, mt):
            # diag block for mt sits at local cols [mt*128, mt*128+128)
            dscr = scr.tile([128, 128], F32, name=f"dscr{mt}", tag="dscr")
            nc.vector.tensor_mul(dscr, s_ps[:, ts(mt, 128)], eye_t)
            nc.vector.tensor_reduce(
                out=diag_sb[:, mt : mt + 1],
                in_=dscr,
                axis=mybir.AxisListType.X,
                op=mybir.AluOpType.add,
            )

        def do_exp(s_ps, lo, hi, g, slot, tag):
            e_t = expp.tile([128, hi - lo], BF16, name=f"e{tag}", tag="exp")
            nc.scalar.activation(
                e_t,
                s_ps[:, lo:hi],
                mybir.ActivationFunctionType.Exp,
                bias=bias_t,
                scale=act_scale,
                accum_out=rowr[:, g, slot : slot + 1],
            )
            return e_t

        for g in range(NG):
            colacc = cap.tile([128, GW], BF16, name=f"cacc{g}", tag="cacc")
            for mt in range(MT):
                s_ps = psum.tile([128, GW], F32, name=f"s{g}_{mt}", tag="spsum")
                first_split = g == 0 and mt == 0
                last_split = g == NG - 1 and mt == MT - 1
                if first_split or last_split:
                    mm_halves(s_ps, g, mt)
                else:
                    mm_full(s_ps, g, mt)
                if g == 0:
                    do_diag(s_ps, mt)
                if first_split:
                    eA = do_exp(s_ps, 0, HW, g, 0, "0A")
                    eB = do_exp(s_ps, HW, GW, g, MT, "0B")
                    nc.vector.tensor_copy(colacc[:, 0:HW], eA)
                    nc.vector.tensor_copy(colacc[:, HW:GW], eB)
                elif last_split:
                    eL = do_exp(s_ps, 0, HW, g, MT - 1, "7L")
                    nc.vector.tensor_add(colacc[:, 0:HW], colacc[:, 0:HW], eL)
                    nc.sync.dma_start(colsum_d[g][:, 0:HW], colacc[:, 0:HW])
                    eR = do_exp(s_ps, HW, GW, g, MT, "7R")
                    nc.vector.tensor_add(colacc[:, HW:GW], colacc[:, HW:GW], eR)
                    nc.sync.dma_start(colsum_d[g][:, HW:GW], colacc[:, HW:GW])
                    nc.sync.dma_start(rowr_d, rowr)
                else:
                    e_t = do_exp(s_ps, 0, GW, g, mt, str((g, mt)))
                    if mt == 0:
                        nc.vector.tensor_copy(colacc, e_t)
                    else:
                        nc.vector.tensor_add(colacc, colacc, e_t)
            if g < NG - 1:
                nc.sync.dma_start(colsum_d[g], colacc)
            if g == 0:
                # diag is complete after group 0; stream it out early.
                nc.sync.dma_start(diag_d, diag_sb)

    nc.compile()
    return nc


def _prep_inputs(img, txt, scale):
    fp8 = ml_dtypes.float8_e4m3fn
    eye = ((scale / (PRE * PRE)) * np.eye(128)).astype(np.float32)
    in_maps = []
    for c in range(NC):
        A = (PRE * img[c * M_LOC : (c + 1) * M_LOC]).astype(fp8)   # [1024, 512]
        # at[p, mt, kc, ko, j] = A[mt*128+j, kc*256+ko*128+p]
        at = np.ascontiguousarray(
            A.T.reshape(KC, 2, 128, MT, 128).transpose(2, 3, 0, 1, 4)
        )                                                          # [128, MT, KC, 2, 128]
        tr = np.roll(txt, -c * M_LOC, axis=0)                      # local col j -> global (j + c*1024) % N
        B = (PRE * tr).astype(fp8)                                 # [8192, 512]
        bt = np.ascontiguousarray(
            B.T.reshape(KC, 2, 128, NG, GW).transpose(3, 0, 2, 1, 4)
        )                                                          # [NG, KC, 128, 2, GW]
        in_maps.append({"at_in": at, "bt_in": bt, "eye_in": eye})
    return in_maps


def kernel(image_features, text_features, logit_scale):
    global LAST_RESULTS
    img = np.ascontiguousarray(np.asarray(image_features, dtype=np.float32))
    txt = np.ascontiguousarray(np.asarray(text_features, dtype=np.float32))
    scale = float(np.asarray(logit_scale))
    shift = 0.5 * scale

    key = (scale, MM_W)
    if key not in _CACHE:
        _CACHE[key] = _build(scale, shift, MM_W)
    nc = _CACHE[key]

    in_maps = _prep_inputs(img, txt, scale)
    res = None
    last_err = None
    for _attempt in range(3):
        try:
            res = run_bass_kernel_spmd(nc, in_maps, core_ids=list(range(NC)))
            break
        except Exception as e:  # transient NRT/device hiccups: retry
            last_err = e
    if res is None:
        raise last_err
    LAST_RESULTS = res

    colsum_tot = np.zeros(N, dtype=np.float64)
    lse_rows = []
    diags = []
    for c, r in enumerate(res.results):
        rr = r["rowr_out"].astype(np.float64)                       # [128, NG, RS]
        # rowsum partials per (p, mt): slot mt for each group, plus the
        # extra slot RS-1 holding g0/mt0's right half and g3/mt7's right
        # half respectively.
        per_mt = rr[:, :, :MT].sum(axis=1)                          # [128, MT]
        per_mt[:, 0] += rr[:, 0, MT]                                # g0 mt0 B-half
        per_mt[:, MT - 1] += rr[:, NG - 1, MT]                      # g3 mt7 R-half
        lse_rows.append(shift + np.log(per_mt.T.reshape(-1)))       # row = mt*128 + p
        diags.append(r["diag_out"].astype(np.float64).T.reshape(-1))
        colsum_tot += np.roll(
            r["colsum_out"].astype(np.float64).sum(axis=1).reshape(-1), c * M_LOC
        )
    lse_row = np.concatenate(lse_rows)
    diag = np.concatenate(diags)
    lse_col = shift + np.log(colsum_tot)

    loss = 0.5 * (np.mean(lse_row - diag) + np.mean(lse_col - diag))
    return np.float32(loss)
